# revision 33
# baseline (speedup 1.0000x reference)
"""DeepseekV2 MLA attention (B=2, S=2048, HID=4096, H=32, QK=192, VD=128)
on 8 trn2 NeuronCores — head-sharded tensor parallel.

Sharding: core c computes latents (q_a / kv_a + rope) for global token slab
c (batch c//4, quarter c%4), AllGathers the bf16 latents across all 8 cores,
then runs full causal attention for ITS 4 global heads [4c, 4c+4) over both
batches (processed sequentially). Attention outputs are AllToAll'd back to
token slabs and each core runs the full o_proj for its 512 tokens.

This removes the 4x kv_b replication of the batch-sharded layout and all
causal padding waste (each query block i only visits its 2i+2 visible key
blocks — identical static program on every core).

Precision: weights + collective transport in bf16, scores q/k in float32r,
PSUM accumulation f32, probs bf16.
"""
import sys

sys.path.insert(0, "/opt/trn_rl_repo")

import numpy as np
import ml_dtypes
import concourse.bass as bass  # noqa: F401
from concourse import bacc
import concourse.mybir as mybir
import concourse.tile as tile
from concourse.bass_utils import run_bass_kernel_spmd

# ---- problem constants (hardcoded per contract) ----
B, S, HID = 2, 2048, 4096
H, NOPE, ROPE, VD = 32, 128, 64, 128
QK = NOPE + ROPE          # 192
QLR, KVLR = 1536, 512
EPS = 1e-6
SCALE = QK ** -0.5

P = 128
TOKQ = 512                # latent tokens per core (global slab)
HL = 4                    # heads per core
NKB = S // P              # 16 key blocks per batch
NQB = S // 256            # 8 query blocks of 256
NEG = np.float32(-1e32)

bf = mybir.dt.bfloat16
fr = mybir.dt.float32r
f32 = mybir.dt.float32

_CACHED = {}

KNOBS = dict(emit_cc=True, tril_gpsimd=False, pss_bufs=3, psA_bufs=2, psd_bufs=1)


def _build_nc():
    key = tuple(sorted(KNOBS.items()))
    if key in _CACHED:
        return _CACHED[key]
    nc = bacc.Bacc("TRN2", target_bir_lowering=False, debug=False, num_devices=8)

    def din(name, shape, dt=bf):
        return nc.dram_tensor(name, shape, dt, kind="ExternalInput").ap()

    hsq = din("hsq_T", [HID, TOKQ])                 # my token slab, feature-major
    wqa = din("wq_a_T", [HID, QLR])
    wqb = din("wq_b_T", [QLR, HL * QK])             # 4x nope(128) then 2x rope-pair(128)
    wkva = din("wkv_a_T", [HID, KVLR + ROPE])
    wkvb = din("wkv_b_T", [KVLR, HL * (NOPE + VD)])
    wo = din("wo_T", [H * VD, HID])
    cc2k_in = din("cc2k", [ROPE, TOKQ], f32)
    ss2k_in = din("ss2k", [ROPE, TOKQ], f32)
    cc2q_in = din("cc2q", [2 * ROPE, S], f32)
    ss2q_in = din("ss2q", [2 * ROPE, S], f32)
    perm_in = din("perm128", [P, P], bf)
    mtril = din("mask_tril", [256, 256], f32)
    ones_in = din("ones_f", [P, P], f32)
    onesb_in = din("ones_b", [P, P], bf)
    outT = nc.dram_tensor("out_T", [HID, TOKQ], f32, kind="ExternalOutput").ap()

    # collective buffers (DRAM). inputs Local, outputs Shared.
    agk_in = nc.dram_tensor("agk_in", [KVLR + ROPE, TOKQ], bf).ap()
    agk_out = nc.dram_tensor("agk_out", [8, KVLR + ROPE, TOKQ], bf,
                             addr_space="Shared").ap()
    agq_in = nc.dram_tensor("agq_in", [QLR, TOKQ], bf).ap()
    agq_out = nc.dram_tensor("agq_out", [8, QLR, TOKQ], bf,
                             addr_space="Shared").ap()
    a2a_in = nc.dram_tensor("a2a_in", [8, HL * VD, TOKQ], bf).ap()
    a2a_out = nc.dram_tensor("a2a_out", [8, HL * VD, TOKQ], bf).ap()

    G8 = [[0, 1, 2, 3, 4, 5, 6, 7]]
    QC = QLR // P             # 12
    KC = KVLR // P            # 4
    HC = HID // P             # 32

    wkvb4 = wkvb.rearrange("(lc p) (hh c) -> p lc hh c", p=P, c=NOPE + VD)

    def emit_cc(kind, in_ap, out_ap):
        if KNOBS["emit_cc"]:
            nc.gpsimd.collective_compute(
                kind, mybir.AluOpType.bypass, replica_groups=G8,
                ins=[in_ap.opt()], outs=[out_ap.opt()])
        else:
            # sim-only stand-in: DMA copies that preserve the dependency
            # structure (and rough byte cost) of the collective.
            n = out_ap.shape[0]
            for s in range(n):
                src = in_ap[s] if list(in_ap.shape) == list(out_ap.shape) \
                    else in_ap
                nc.gpsimd.dma_start(out_ap[s], src)

    with tile.TileContext(nc) as tc:
        with tc.tile_pool(name="persist", bufs=1) as persist:
            tril_sb = persist.tile([P, 2, 256], f32)
            perm_sb = persist.tile([P, P], bf)
            ones_fr128 = persist.tile([P, 1], fr)   # col of ones (sums stat.)
            ones_bf128 = persist.tile([P, 1], bf)   # col of ones (denom stat.)
            eps_t = persist.tile([1, 1], f32)
            nc.vector.memset(eps_t[:], EPS)
            nc.scalar.dma_start(tril_sb[:], mtril.rearrange("(kb p) q -> p kb q", p=P))
            nc.scalar.dma_start(perm_sb[:], perm_in)
            nc.scalar.dma_start(ones_fr128[:], ones_in[:, 0:1].bitcast(fr))
            nc.scalar.dma_start(ones_bf128[:], onesb_in[:, 0:1])

            # ================= Phase A: my token slab's latents =================
            with tc.tile_pool(name="pa", bufs=1) as pa, \
                 tc.tile_pool(name="paw", bufs=2) as paw, \
                 tc.tile_pool(name="pasq", bufs=3) as pasq, \
                 tc.tile_pool(name="pars", bufs=4) as pars, \
                 tc.tile_pool(name="pa_ps", bufs=5, space="PSUM") as paps, \
                 tc.tile_pool(name="pa_ps2", bufs=2, space="PSUM") as paps2:
                hsq_sb = pa.tile([P, HC, TOKQ], bf)
                cc2k = pa.tile([ROPE, TOKQ], f32)
                ss2k = pa.tile([ROPE, TOKQ], f32)
                nc.scalar.dma_start(cc2k[:], cc2k_in)
                nc.scalar.dma_start(ss2k[:], ss2k_in)

                # ---- kv pass: 4 latent chunks + rope, fused normalize ----
                # interleave input/weight DMA in 8-chunk slices so the first
                # matmuls start ~4x earlier than a monolithic load
                wkv_sb = paw.tile([P, HC, KVLR + ROPE], bf, tag="wkv",
                                  bufs=1, name="wkv")
                hsq_r = hsq.rearrange("(p hc) t -> p hc t", hc=HC)
                wkv_r = wkva.rearrange("(p hc) m -> p hc m", hc=HC)
                for ch in range(8):
                    csl = slice(ch * 4, (ch + 1) * 4)
                    nc.sync.dma_start(hsq_sb[:, csl, :], hsq_r[:, csl, :])
                    nc.sync.dma_start(wkv_sb[:, csl, :], wkv_r[:, csl, :])
                kps = [paps.tile([P, TOKQ], f32, tag="pa", name=f"kv{i}")
                       for i in range(KC)]
                rotps = paps.tile([ROPE, TOKQ], f32, tag="pa", name="rot")
                sums_k = paps2.tile([1, TOKQ], f32, tag="sums", name="sums_k")
                for hc in range(HC):
                    for i in range(KC):
                        nc.tensor.matmul(
                            kps[i][:], wkv_sb[:, hc, i * P:(i + 1) * P],
                            hsq_sb[:, hc, :],
                            start=(hc == 0), stop=(hc == HC - 1))
                    nc.tensor.matmul(
                        rotps[:], wkv_sb[:, hc, KVLR:], hsq_sb[:, hc, :],
                        start=(hc == 0), stop=(hc == HC - 1))
                for i in range(KC):
                    sq = pasq.tile([P, TOKQ], fr, tag="sq", name=f"sqk{i}")
                    nc.scalar.activation(sq[:], kps[i][:],
                                         mybir.ActivationFunctionType.Square)
                    nc.tensor.matmul(sums_k[:], ones_fr128[:], sq[:],
                                     start=(i == 0), stop=(i == KC - 1))
                rsk_sq = pars.tile([1, TOKQ], f32, tag="rs", name="rsk_sq")
                nc.scalar.activation(rsk_sq[:], sums_k[:],
                                     mybir.ActivationFunctionType.Sqrt,
                                     bias=eps_t[:], scale=1.0 / KVLR)
                rsk_row = pars.tile([1, TOKQ], f32, tag="rs", name="rsk_row")
                nc.vector.reciprocal(rsk_row[:], rsk_sq[:])
                rskb = pars.tile([P, TOKQ], f32, tag="rsb", name="rskb")
                nc.gpsimd.partition_broadcast(rskb[:], rsk_row[:])
                k_stage = pa.tile([P, KC, TOKQ], bf)
                with nc.allow_low_precision(reason="bf16 latent transport"):
                    for i in range(KC):
                        nc.vector.tensor_tensor(
                            out=k_stage[:, i, :], in0=kps[i][:], in1=rskb[:],
                            op=mybir.AluOpType.mult)
                nc.sync.dma_start(
                    agk_in[0:KVLR, :].rearrange("(p lc) t -> p lc t", lc=KC),
                    k_stage[:])
                # rope on k_rot (no norm): partner swap via sbuf-sbuf DMA
                kr_raw = pars.tile([ROPE, TOKQ], f32, tag="kr", name="kr_raw")
                nc.vector.tensor_copy(kr_raw[:], rotps[:])
                kr_par = pars.tile([ROPE, TOKQ], f32, tag="kr", name="kr_par")
                HR = ROPE // 2
                nc.sync.dma_start(kr_par[:HR, :], kr_raw[HR:, :])
                nc.sync.dma_start(kr_par[HR:, :], kr_raw[:HR, :])
                t1 = pars.tile([ROPE, TOKQ], f32, tag="kr", name="kr_t1")
                nc.vector.tensor_tensor(out=t1[:], in0=kr_raw[:], in1=cc2k[:],
                                        op=mybir.AluOpType.mult)
                t2 = pars.tile([ROPE, TOKQ], f32, tag="kr", name="kr_t2")
                nc.vector.tensor_tensor(out=t2[:], in0=kr_par[:], in1=ss2k[:],
                                        op=mybir.AluOpType.mult)
                krn = pars.tile([ROPE, TOKQ], bf, tag="kr", name="krn")
                with nc.allow_low_precision(reason="bf16 latent transport"):
                    nc.vector.tensor_tensor(out=krn[:], in0=t1[:], in1=t2[:],
                                            op=mybir.AluOpType.add)
                nc.sync.dma_start(agk_in[KVLR:, :], krn[:])
                emit_cc("AllGather", agk_in, agk_out)

                # ---- q passes: 12 chunks in 3 groups of 4 ----
                q_stage = pa.tile([P, QC, TOKQ], bf)
                sums_q = paps2.tile([1, TOKQ], f32, tag="sums", name="sums_q")
                for grp in range(6):
                    wq_sb = paw.tile([P, HC, 2 * P], bf, tag="w",
                                     name=f"wq{grp}")
                    nc.sync.dma_start(
                        wq_sb[:],
                        wqa[:, grp * 2 * P:(grp + 1) * 2 * P]
                        .rearrange("(p hc) m -> p hc m", hc=HC))
                    qps = [paps.tile([P, TOKQ], f32, tag="pa",
                                     name=f"q{grp}_{i}") for i in range(2)]
                    for hc in range(HC):
                        for i in range(2):
                            nc.tensor.matmul(
                                qps[i][:], wq_sb[:, hc, i * P:(i + 1) * P],
                                hsq_sb[:, hc, :],
                                start=(hc == 0), stop=(hc == HC - 1))
                    for i in range(2):
                        ob = grp * 2 + i
                        sq = pasq.tile([P, TOKQ], fr, tag="sq",
                                       name=f"sqq{ob}")
                        nc.scalar.activation(
                            sq[:], qps[i][:],
                            mybir.ActivationFunctionType.Square)
                        nc.tensor.matmul(sums_q[:], ones_fr128[:], sq[:],
                                         start=(ob == 0), stop=(ob == QC - 1))
                        with nc.allow_low_precision(reason="bf16 latent transport"):
                            nc.vector.tensor_copy(q_stage[:, ob, :], qps[i][:])
                rsq_sq = pars.tile([1, TOKQ], f32, tag="rs", name="rsq_sq")
                nc.scalar.activation(rsq_sq[:], sums_q[:],
                                     mybir.ActivationFunctionType.Sqrt,
                                     bias=eps_t[:], scale=1.0 / QLR)
                rsq_row = pars.tile([1, TOKQ], f32, tag="rs", name="rsq_row")
                nc.vector.reciprocal(rsq_row[:], rsq_sq[:])
                rsqb = pars.tile([P, TOKQ], f32, tag="rsb", name="rsqb")
                nc.gpsimd.partition_broadcast(rsqb[:], rsq_row[:])
                with nc.allow_low_precision(reason="bf16 latent transport"):
                    for ob in range(QC):
                        nc.vector.tensor_tensor(
                            out=q_stage[:, ob, :], in0=q_stage[:, ob, :],
                            in1=rsqb[:], op=mybir.AluOpType.mult)
                nc.scalar.dma_start(
                    agq_in.rearrange("(p lc) t -> p lc t", lc=QC), q_stage[:])
                emit_cc("AllGather", agq_in, agq_out)

            # ================= Phase B: 4 heads x 2 batches =================
            with tc.tile_pool(name="lat", bufs=1) as lat, \
                 tc.tile_pool(name="hb", bufs=2) as hb, \
                 tc.tile_pool(name="hb1", bufs=1) as hb1, \
                 tc.tile_pool(name="hbq", bufs=2) as hbq, \
                 tc.tile_pool(name="hb4", bufs=4) as hb4, \
                 tc.tile_pool(name="wb", bufs=2) as wb, \
                 tc.tile_pool(name="rt", bufs=2) as rt, \
                 tc.tile_pool(name="ps_A", bufs=KNOBS["psA_bufs"], space="PSUM") as psA, \
                 tc.tile_pool(name="ps_s", bufs=KNOBS["pss_bufs"], space="PSUM") as ps_s, \
                 tc.tile_pool(name="ps_o", bufs=2, space="PSUM") as ps_o, \
                 tc.tile_pool(name="ps_d", bufs=KNOBS["psd_bufs"], space="PSUM") as ps_d:
                for bb in range(B):
                    # this batch's gathered latents (tag ring 1 => batches
                    # serialize on the same SBUF)
                    if bb == 0:
                        cc2q = lat.tile([2 * ROPE, S], f32, tag="ccq")
                        ss2q = lat.tile([2 * ROPE, S], f32, tag="ssq")
                        nc.gpsimd.dma_start(cc2q[:], cc2q_in)
                        nc.gpsimd.dma_start(ss2q[:], ss2q_in)
                    q_lat = lat.tile([P, QC, S], bf, tag="qlat",
                                     name=f"qlat{bb}")
                    # k-side tiles live in the persist pool (below phase A
                    # SBUF) so batch-0 loads land the moment the kv
                    # AllGather finishes; double buffered so batch 1
                    # prefetches during batch 0 compute
                    k_lat = persist.tile([P, KC, S], bf, tag="klat", bufs=2,
                                         name=f"klat{bb}")
                    # k_rot duplicated on partitions 64:128 so odd heads'
                    # q_rot slice (base partition 64) has a matching
                    # stationary base.
                    krot_bf = persist.tile([P, S], bf, tag="krbf", bufs=2,
                                           name=f"krbf{bb}")
                    # k-side loads on the sync ring (ready first)...
                    for gp in range(4):
                        sl = slice(gp * TOKQ, (gp + 1) * TOKQ)
                        nc.sync.dma_start(
                            k_lat[:, :, sl],
                            agk_out[4 * bb + gp, 0:KVLR, :]
                            .rearrange("(p lc) t -> p lc t", lc=KC))
                        nc.sync.dma_start(krot_bf[0:ROPE, sl],
                                          agk_out[4 * bb + gp, KVLR:, :])
                    nc.sync.dma_start(krot_bf[ROPE:, :], krot_bf[0:ROPE, :])
                    # ...q-side loads on the gpsimd ring so they don't
                    # head-block the k-path weight DMAs while the q
                    # AllGather is still in flight
                    for gp in range(4):
                        sl = slice(gp * TOKQ, (gp + 1) * TOKQ)
                        nc.gpsimd.dma_start(
                            q_lat[:, :, sl],
                            agq_out[4 * bb + gp, :, :]
                            .rearrange("(p lc) t -> p lc t", lc=QC))

                    # v for all 4 local heads, token-major
                    vg = hb1.tile([P, NKB, HL * VD], bf, tag="vg",
                                  name=f"vg{bb}")
                    wv = persist.tile([P, KC, HL, VD], bf, tag="wv", bufs=2,
                                      name=f"wv{bb}")
                    for lc in range(KC):
                        nc.sync.dma_start(wv[:, lc], wkvb4[:, lc, :, NOPE:])
                    for tb in range(NKB):
                        psv = psA.tile([P, HL * VD], f32, tag="A",
                                       name=f"v{bb}_{tb}")
                        for lc in range(KC):
                            nc.tensor.matmul(
                                psv[:], k_lat[:, lc, tb * P:(tb + 1) * P],
                                wv[:, lc].rearrange("p a b -> p (a b)"),
                                start=(lc == 0), stop=(lc == KC - 1))
                        with nc.allow_low_precision(reason="bf16 v"):
                            nc.vector.tensor_copy(vg[:, tb, :], psv[:])

                    for h in range(HL):
                        pr, side = divmod(h, 2)
                        # k_pass for this head [NOPE, S] (fr for scores)
                        wk = wb.tile([P, KC, NOPE], bf, tag="wk",
                                     name=f"wk{bb}_{h}")
                        nc.sync.dma_start(wk[:], wkvb4[:, :, h, :NOPE])
                        kh_sb = hb.tile([NOPE, 4, TOKQ], bf, tag="khead",
                                        name=f"kh{bb}_{h}")
                        for tt in range(4):
                            psk = psA.tile([NOPE, TOKQ], f32, tag="A",
                                           name=f"k{bb}_{h}_{tt}")
                            for lc in range(KC):
                                nc.tensor.matmul(
                                    psk[:], wk[:, lc, :],
                                    k_lat[:, lc, tt * TOKQ:(tt + 1) * TOKQ],
                                    start=(lc == 0), stop=(lc == KC - 1))
                            with nc.allow_low_precision(reason="bf16 scores"):
                                nc.vector.tensor_copy(kh_sb[:, tt, :], psk[:])
                        # q nope for this head
                        wqn = wb.tile([P, QC, NOPE], bf, tag="wqn",
                                      name=f"wqn{bb}_{h}")
                        nc.sync.dma_start(
                            wqn[:],
                            wqb[:, h * NOPE:(h + 1) * NOPE]
                            .rearrange("(lc p) m -> p lc m", p=P))
                        q_pass = hbq.tile([NOPE, 4, TOKQ], bf, tag="qpass",
                                          name=f"qp{bb}_{h}")
                        for tt in range(4):
                            psq = psA.tile([NOPE, TOKQ], f32, tag="A",
                                           name=f"qn{bb}_{h}_{tt}")
                            for lc in range(QC):
                                nc.tensor.matmul(
                                    psq[:], wqn[:, lc, :],
                                    q_lat[:, lc, tt * TOKQ:(tt + 1) * TOKQ],
                                    start=(lc == 0), stop=(lc == QC - 1))
                            with nc.allow_low_precision(reason="bf16 scores"):
                                nc.vector.tensor_copy(q_pass[:, tt, :], psq[:])
                        # q rope for head PAIR (computed at even h)
                        if side == 0:
                            wqr = wb.tile([P, QC, P], bf, tag="wqr",
                                          name=f"wqr{bb}_{pr}")
                            nc.sync.dma_start(
                                wqr[:],
                                wqb[:, HL * NOPE + pr * P:HL * NOPE + (pr + 1) * P]
                                .rearrange("(lc p) m -> p lc m", p=P))
                            qr_pair = hbq.tile([P, 4, TOKQ], bf, tag="qr",
                                               bufs=1, name=f"qr{bb}_{pr}")
                            q_rot = hbq.tile([P, 4, TOKQ], bf, tag="qrot",
                                             bufs=1, name=f"qrot{bb}_{pr}")
                            for tt in range(4):
                                psr = psA.tile([P, TOKQ], f32, tag="A",
                                               name=f"qr{bb}_{pr}_{tt}")
                                for lc in range(QC):
                                    nc.tensor.matmul(
                                        psr[:], wqr[:, lc, :],
                                        q_lat[:, lc, tt * TOKQ:(tt + 1) * TOKQ],
                                        start=(lc == 0), stop=(lc == QC - 1))
                                with nc.allow_low_precision(reason="bf16 scores"):
                                    nc.vector.tensor_copy(qr_pair[:, tt, :], psr[:])
                                psp = psA.tile([P, TOKQ], f32, tag="A",
                                               name=f"qperm{bb}_{pr}_{tt}")
                                nc.tensor.matmul(psp[:], perm_sb[:],
                                                 qr_pair[:, tt, :],
                                                 start=True, stop=True)
                                sl = slice(tt * TOKQ, (tt + 1) * TOKQ)
                                r1 = rt.tile([P, TOKQ], f32, tag="r",
                                             name=f"r1_{bb}_{pr}_{tt}")
                                nc.vector.tensor_tensor(
                                    out=r1[:], in0=qr_pair[:, tt, :],
                                    in1=cc2q[:, sl], op=mybir.AluOpType.mult)
                                r2 = rt.tile([P, TOKQ], f32, tag="r",
                                             name=f"r2_{bb}_{pr}_{tt}")
                                nc.vector.tensor_tensor(
                                    out=r2[:], in0=psp[:], in1=ss2q[:, sl],
                                    op=mybir.AluOpType.mult)
                                with nc.allow_low_precision(reason="bf16 scores"):
                                    nc.vector.tensor_tensor(
                                        out=q_rot[:, tt, :], in0=r1[:],
                                        in1=r2[:], op=mybir.AluOpType.add)
                        qro = (h % 2) * ROPE   # partition offset in pair tile

                        # ---- causal attention: block i sees kb 0..2i+1 ----
                        attn_sb = hb.tile([VD, S], bf, tag="attn",
                                          name=f"at{bb}_{h}")
                        for ip in range(NQB // 2):   # i pairs (2ip, 2ip+1)
                            pso = ps_o.tile([VD, 2 * 256], f32, tag="o",
                                            name=f"o{bb}_{h}_{ip}")
                            psd = ps_d.tile([1, 2 * 256], f32, tag="d",
                                            name=f"d{bb}_{h}_{ip}")
                            for ih in range(2):
                                i = 2 * ip + ih
                                tt, half = divmod(i, 2)
                                qslt = slice(half * 256, (half + 1) * 256)
                                osl = slice(ih * 256, (ih + 1) * 256)
                                nkb = 2 * i + 2

                                def emit_dp(probs, kb, osl=osl, nkb=nkb, h=h):
                                    nc.tensor.matmul(
                                        psd[:, osl], ones_bf128[:], probs[:],
                                        start=(kb == 0), stop=(kb == nkb - 1))
                                    nc.tensor.matmul(
                                        pso[:, osl],
                                        vg[:, kb, h * VD:(h + 1) * VD],
                                        probs[:],
                                        start=(kb == 0), stop=(kb == nkb - 1))

                                pend = None
                                for kp in range(i + 1):
                                    pss = ps_s.tile([P, 512], f32, tag="s",
                                                    name=f"s{bb}_{h}_{i}_{kp}")
                                    for kh in range(2):
                                        kb = 2 * kp + kh
                                        csl = slice(kh * 256, (kh + 1) * 256)
                                        kt, ko = divmod(kb, 4)
                                        nc.tensor.matmul(
                                            pss[:, csl],
                                            kh_sb[:, kt, ko * P:(ko + 1) * P],
                                            q_pass[:, tt, qslt],
                                            start=True, stop=False)
                                        nc.tensor.matmul(
                                            pss[:, csl],
                                            krot_bf[qro:qro + ROPE,
                                                    kb * P:(kb + 1) * P],
                                            q_rot[qro:qro + ROPE, tt, qslt],
                                            start=False, stop=True)
                                        if kp == i:
                                            eng = (nc.gpsimd
                                                   if KNOBS["tril_gpsimd"]
                                                   else nc.vector)
                                            eng.tensor_tensor(
                                                out=pss[:, csl],
                                                in0=pss[:, csl],
                                                in1=tril_sb[:, kh, :],
                                                op=mybir.AluOpType.add)
                                    # one exp over both kb halves: the ~200ns
                                    # fixed ACT overhead amortizes 2x
                                    probs = hb4.tile([P, 512], bf,
                                                     tag="probs", bufs=4)
                                    with nc.allow_low_precision(reason="bf16 probs"):
                                        nc.scalar.activation(
                                            probs[:], pss[:],
                                            mybir.ActivationFunctionType.Exp,
                                            scale=SCALE)
                                    # psd/pso for the PREVIOUS pair — keeps
                                    # the exp chain off PE's critical path
                                    if pend is not None:
                                        for kh in range(2):
                                            pb, kb = pend
                                            emit_dp(pb[:, kh * 256:(kh + 1) * 256],
                                                    kb + kh)
                                    pend = (probs, 2 * kp)
                                for kh in range(2):
                                    pb, kb = pend
                                    emit_dp(pb[:, kh * 256:(kh + 1) * 256],
                                            kb + kh)
                            # normalize pair (queries [2ip*256, (2ip+2)*256))
                            rec = hb4.tile([1, 512], f32, tag="rec", bufs=2)
                            nc.vector.reciprocal(rec[:], psd[:])
                            recb = hb4.tile([P, 512], f32, tag="recb", bufs=2)
                            nc.gpsimd.partition_broadcast(recb[:], rec[:])
                            asl = slice(ip * 512, (ip + 1) * 512)
                            with nc.allow_low_precision(reason="bf16 attn"):
                                nc.vector.tensor_tensor(
                                    out=attn_sb[:, asl], in0=pso[:],
                                    in1=recb[:VD, :], op=mybir.AluOpType.mult)
                        # ship to a2a slabs (batch bb quarters)
                        for qtr in range(4):
                            nc.scalar.dma_start(
                                a2a_in[4 * bb + qtr,
                                       h * VD:(h + 1) * VD, :],
                                attn_sb[:, qtr * TOKQ:(qtr + 1) * TOKQ])
                emit_cc("AllToAll", a2a_in, a2a_out)

        # ================= Phase C: o_proj on my token slab =================
        with tc.tile_pool(name="pc", bufs=3) as pc, \
             tc.tile_pool(name="pc_ps", bufs=8, space="PSUM") as pcps:
            # all 32 heads' attn for my slab, loaded once (8 descriptors)
            attn_all = pc.tile([P, H, TOKQ], bf, tag="attn_all", bufs=1)
            for fg in range(8):
                nc.scalar.dma_start(
                    attn_all[:, 4 * fg:4 * (fg + 1), :],
                    a2a_out[fg].rearrange("(f p) t -> p f t", p=P))
            for hz in range(8):   # half-pz chunks of 4 output row-blocks
                psums = [pcps.tile([P, TOKQ], f32, tag="po",
                                   name=f"po{hz}_{i}") for i in range(4)]
                for fg in range(8):   # groups of 4 global heads
                    wot4 = pc.tile([P, 4, 4, P], bf, tag="wo", bufs=2,
                                   name=f"wo{hz}_{fg}")
                    nc.sync.dma_start(
                        wot4[:],
                        wo[fg * 512:(fg + 1) * 512,
                           hz * 512:(hz + 1) * 512]
                        .rearrange("(f p) (i c) -> p f i c", p=P, c=P))
                    for f in range(4):
                        fc = 4 * fg + f
                        for i in range(4):
                            nc.tensor.matmul(
                                psums[i][:], wot4[:, f, i, :],
                                attn_all[:, fc, :],
                                start=(fc == 0), stop=(fc == H - 1))
                for i in range(4):
                    osb = pc.tile([P, TOKQ], f32, tag="osb")
                    nc.vector.tensor_copy(osb[:], psums[i][:])
                    nc.sync.dma_start(
                        outT[(hz * 4 + i) * P:(hz * 4 + i + 1) * P, :], osb[:])

    nc.finalize()
    _CACHED[key] = nc
    return nc


def _prep_in_maps(hidden_states, cos, sin, q_a_w, q_a_ln_w, q_b_w, kv_a_w,
                  kv_a_ln_w, kv_b_w, o_w):
    bft = ml_dtypes.bfloat16
    hs = np.asarray(hidden_states, np.float32)
    cos = np.asarray(cos, np.float32)
    sin = np.asarray(sin, np.float32)
    rp = np.concatenate([np.arange(0, ROPE, 2), np.arange(1, ROPE, 2)])

    def pmajor(m):
        # rows (hc p) -> (p hc): partition-contiguous DMA runs
        return np.ascontiguousarray(
            m.reshape(HID // P, P, -1).transpose(1, 0, 2).reshape(m.shape))

    wqa_T = pmajor(np.ascontiguousarray(
        np.asarray(q_a_w, np.float32).T)).astype(bft)
    qb = (np.asarray(q_b_w, np.float32)
          * np.asarray(q_a_ln_w, np.float32)[None, :]).reshape(H, QK, QLR)
    qb = np.concatenate([qb[:, :NOPE], qb[:, NOPE:][:, rp]], axis=1)
    kva = np.asarray(kv_a_w, np.float32).copy()
    kva[KVLR:] = kva[KVLR:][rp]
    wkva_T = pmajor(np.ascontiguousarray(kva.T)).astype(bft)
    kvb = (np.asarray(kv_b_w, np.float32)
           * np.asarray(kv_a_ln_w, np.float32)[None, :]).reshape(
               H, NOPE + VD, KVLR)
    wo_T = np.ascontiguousarray(np.asarray(o_w, np.float32).T).astype(bft)
    ones_f = np.ones((P, P), np.float32)
    ones_b = np.ones((P, P), bft)
    tr = np.tril(np.full((256, 256), NEG, np.float32), -1)
    pm64 = np.zeros((ROPE, ROPE), np.float32)
    for i_ in range(ROPE):
        pm64[(i_ + ROPE // 2) % ROPE, i_] = 1.0
    pm128 = np.zeros((P, P), np.float32)
    pm128[:ROPE, :ROPE] = pm64
    pm128[ROPE:, ROPE:] = pm64
    pm128 = pm128.astype(bft)

    # q-side trig (identical across batches: positions 0..S-1), rows
    # duplicated for head pairs
    assert np.array_equal(cos[0], cos[1]) and np.array_equal(sin[0], sin[1])
    cq = np.ascontiguousarray(cos[0].T)          # [32, S]
    sq_ = np.ascontiguousarray(sin[0].T)
    cc2q = np.concatenate([cq, cq, cq, cq], 0)
    ss2q = np.concatenate([-sq_, sq_, -sq_, sq_], 0)

    in_maps = []
    for c in range(8):
        bL, g = divmod(c, 4)
        tsl = slice(TOKQ * g, TOKQ * (g + 1))
        hsq_T = pmajor(np.ascontiguousarray(hs[bL].T[:, tsl])).astype(bft)
        ck = np.ascontiguousarray(cos[bL, tsl].T)
        sk = np.ascontiguousarray(sin[bL, tsl].T)
        cc2k = np.concatenate([ck, ck], 0)
        ss2k = np.concatenate([-sk, sk], 0)
        # wq_b for my 4 heads: 4 nope blocks then 2 rope-pair blocks
        hsel = qb[4 * c:4 * c + 4]               # [4, QK, QLR]
        cols = [hsel[j, :NOPE].T for j in range(HL)]
        for pr2 in range(HL // 2):
            cols.append(np.concatenate(
                [hsel[2 * pr2, NOPE:], hsel[2 * pr2 + 1, NOPE:]], 0).T)
        wqb_T = np.ascontiguousarray(np.concatenate(cols, 1)).astype(bft)
        wkvb_T = np.ascontiguousarray(
            kvb[4 * c:4 * c + 4].transpose(2, 0, 1).reshape(
                KVLR, HL * (NOPE + VD))).astype(bft)
        in_maps.append({
            "hsq_T": hsq_T,
            "wq_a_T": wqa_T, "wq_b_T": wqb_T,
            "wkv_a_T": wkva_T, "wkv_b_T": wkvb_T, "wo_T": wo_T,
            "cc2k": cc2k, "ss2k": ss2k,
            "cc2q": cc2q, "ss2q": ss2q,
            "perm128": pm128,
            "mask_tril": tr,
            "ones_f": ones_f,
            "ones_b": ones_b,
        })
    return in_maps


def kernel(**inputs) -> np.ndarray:
    nc = _build_nc()
    in_maps = _prep_in_maps(**inputs)
    res = run_bass_kernel_spmd(nc, in_maps, core_ids=list(range(8)))
    out = np.empty((B, S, HID), np.float32)
    for c in range(8):
        bL, g = divmod(c, 4)
        out[bL, TOKQ * g:TOKQ * (g + 1), :] = res.results[c]["out_T"].T
    return out
